# revision 24
# baseline (speedup 1.0000x reference)
"""EvolveGCN-H Trainium2 kernel (8-core SPMD).

Math (see reference):
  score = X @ p / ||p||; top-128 nodes -> X_tilde -> GRU -> W_new
  deg[n] = 1 + sum_{e: dst=n} w[e]
  dinv = rsqrt(deg) (deg>0 always for real nodes)
  G[n,:] = sum_{e: dst=n, incl self-loop} w[e] * (dinv*X)[src[e], :]
  out = dinv[:,None] * (G @ W_new^T) + b_conv

Device split:
  K1 (heavy): scores, deg/dinv, Xd = dinv*X, AllGather(Xd), edge gather +
      one-hot-matmul scatter into per-dst-window PSUM -> G^T per core.
  host: exact top-k of scores (control plane), X_tilde, tiny GRU -> W_new.
  K2 (small): out = dinv * (G @ W_new^T) + b_conv per core.

Sharding: edges sharded by dst range (6250 dst-nodes per core), node
features 1D-sharded and allgathered after dinv scaling; only output
rows are written by their owning core (no final all-reduce needed).
"""

import math
import os
import sys

import numpy as np

try:
    import concourse.bass as bass
except ImportError:  # container default location
    sys.path.insert(0, "/opt/trn_rl_repo")
    import concourse.bass as bass

import concourse.bacc as bacc
import concourse.tile as tile
from concourse import library_config, mybir
from concourse.bass_utils import run_bass_kernel_spmd

F32 = mybir.dt.float32
I16 = mybir.dt.int16
I32 = mybir.dt.int32
AF = mybir.ActivationFunctionType
ALU = mybir.AluOpType

NCORES = 8
P = 128  # partitions / block size / window width / channels
HALF_SPLIT = 32768  # int16 positive index limit (gather table A/B split)


def _ceil_div(a, b):
    return (a + b - 1) // b


def _round_up(a, b):
    return _ceil_div(a, b) * b


# --------------------------------------------------------------------------
# host-side shard planning (pure index bookkeeping + data staging)
# --------------------------------------------------------------------------
class Plan:
    pass


def plan_shards(X, edge_index, edge_weight, ncores=NCORES, chunk_blocks=32):
    """Bucket edges by dst core / 128-node window / gather-table half, pad to
    SPMD-uniform block counts, and build all per-core staged arrays."""
    N, C = X.shape
    assert C == P
    E = edge_weight.shape[0]
    pl = Plan()
    pl.N, pl.C, pl.E, pl.ncores = N, C, E, ncores
    NPC = _ceil_div(N, ncores)
    NPAD = _round_up(NPC, P)
    NW = NPAD // P
    TROWS = ncores * NPAD
    assert TROWS < 2 * HALF_SPLIT, "table must fit two int16 index halves"
    pl.NPC, pl.NPAD, pl.NW, pl.TROWS = NPC, NPAD, NW, TROWS

    src = np.asarray(edge_index[0], dtype=np.int64)
    dst = np.asarray(edge_index[1], dtype=np.int64)
    w = np.asarray(edge_weight, dtype=np.float32)
    loop = np.arange(N, dtype=np.int64)
    src2 = np.concatenate([src, loop])
    dst2 = np.concatenate([dst, loop])
    w2 = np.concatenate([w, np.ones(N, np.float32)])

    mcore = dst2 // NPC
    dloc = dst2 - mcore * NPC
    srow = (src2 // NPC) * NPAD + (src2 % NPC)  # padded table row of src
    win = dloc // P
    dstrel = (dloc - win * P).astype(np.float32)
    half = (srow >= HALF_SPLIT).astype(np.int64)

    # ---- wpad for degree: rank of each edge within its (core, dloc) group
    key_deg = mcore * NPAD + dloc
    perm = np.argsort(key_deg, kind="stable")
    ks = key_deg[perm]
    starts = np.r_[0, np.flatnonzero(ks[1:] != ks[:-1]) + 1]
    counts = np.diff(np.r_[starts, ks.size])
    rank_sorted = np.arange(ks.size) - np.repeat(starts, counts)
    rank_deg = np.empty(ks.size, np.int64)
    rank_deg[perm] = rank_sorted
    DCAP = _round_up(int(counts.max()), 4)
    pl.DCAP = DCAP
    wpad = np.zeros((ncores, NPAD, DCAP), np.float32)
    wpad[mcore, dloc, rank_deg] = w2
    # wrap rows->partitions: [core, 128, NW*DCAP]
    pl.wpad_w = (
        wpad.reshape(ncores, NW, P, DCAP)
        .transpose(0, 2, 1, 3)
        .reshape(ncores, P, NW * DCAP)
        .copy()
    )

    # ---- edge slot assignment: (core, half, window) groups
    key_slot = (mcore * 2 + half) * NW + win
    perm2 = np.argsort(key_slot, kind="stable")
    ks2 = key_slot[perm2]
    starts2 = np.r_[0, np.flatnonzero(ks2[1:] != ks2[:-1]) + 1]
    counts2 = np.diff(np.r_[starts2, ks2.size])
    rank_sorted2 = np.arange(ks2.size) - np.repeat(starts2, counts2)
    rank_slot = np.empty(ks2.size, np.int64)
    rank_slot[perm2] = rank_sorted2

    cnt = np.zeros((ncores, 2, NW), np.int64)
    np.add.at(cnt, (mcore, half, win), 1)
    blocks = _ceil_div(cnt, P)  # [ncores, 2, NW]
    pl.use_b = TROWS > HALF_SPLIT
    NA = np.maximum(1, blocks[:, 0, :].max(axis=0))  # [NW]
    NB = (
        np.maximum(1, blocks[:, 1, :].max(axis=0))
        if pl.use_b
        else np.zeros(NW, np.int64)
    )
    pl.NA, pl.NB = NA.tolist(), NB.tolist()
    NTA, NTB = int(NA.sum()), int(NB.sum())
    pl.NTA, pl.NTB = NTA, NTB
    NT = NTA + NTB
    pl.NT = NT
    SA, SB = NTA * P, NTB * P
    pl.SA, pl.SB = SA, SB
    STOT = SA + SB
    pl.STOT = STOT
    Aoff = np.r_[0, np.cumsum(NA)][:-1]  # block offset of window w in phase A
    Boff = np.r_[0, np.cumsum(NB)][:-1]
    # slot start of (half, window), same for every core (compiled layout)
    g_start = np.where(
        np.arange(2)[:, None] == 0, Aoff[None, :] * P, SA + Boff[None, :] * P
    )  # [2, NW]
    pos = g_start[half, win] + rank_slot  # global slot of each edge in its core

    idx16 = np.zeros((ncores, STOT), np.int16)
    dstrel_arr = np.zeros((ncores, STOT), np.float32)
    w_arr = np.zeros((ncores, STOT), np.float32)
    idxval = (srow - HALF_SPLIT * half).astype(np.int16)
    idx16[mcore, pos] = idxval
    dstrel_arr[mcore, pos] = dstrel
    w_arr[mcore, pos] = w2

    # 16-wrap the idx array (slot i -> [i%16, i//16]), replicate to 128 parts
    idx_w = idx16.reshape(ncores, STOT // 16, 16).transpose(0, 2, 1)  # [nc,16,S/16]
    pl.idx_w = np.tile(idx_w, (1, P // 16, 1)).copy()  # [nc, 128, S/16]
    # 128-wrap dstrel/w (slot i -> [i%128, i//128])
    pl.dstrel_w = dstrel_arr.reshape(ncores, NT, P).transpose(0, 2, 1).copy()
    pl.w_w = w_arr.reshape(ncores, NT, P).transpose(0, 2, 1).copy()

    # ---- node features, zero-padded per core
    Xp = np.zeros((ncores, NPAD, C), np.float32)
    Xp.reshape(ncores * NPAD, C)[
        (np.arange(N) // NPC) * NPAD + (np.arange(N) % NPC)
    ] = np.asarray(X, np.float32)
    pl.Xp = Xp

    # ---- gather call chunking (compile-time)
    pl.num_queues = 1
    pl.chunk_blocks = chunk_blocks
    calls = []  # (phase, block_start, nblocks)
    for ph, nt in ((0, NTA), (1, NTB)):
        b0 = 0 if ph == 0 else NTA
        for cs in range(0, nt, chunk_blocks):
            calls.append((ph, b0 + cs, min(chunk_blocks, nt - cs)))
    pl.calls = calls
    return pl


# --------------------------------------------------------------------------
# K1 program: scores, dinv, Xd allgather, edge aggregation -> G^T
# --------------------------------------------------------------------------
def build_k1(pl):
    nc = bacc.Bacc(None, num_swdge_queues=pl.num_queues)
    NPAD, NW, NT, DCAP, TROWS = pl.NPAD, pl.NW, pl.NT, pl.DCAP, pl.TROWS
    STOT = pl.STOT

    X_in = nc.dram_tensor("X_in", [NPAD, P], F32, kind="ExternalInput")
    wpad_in = nc.dram_tensor("wpad_in", [P, NW * DCAP], F32, kind="ExternalInput")
    p_in = nc.dram_tensor("p_in", [1, P], F32, kind="ExternalInput")
    iota_in = nc.dram_tensor("iota_in", [P, P], F32, kind="ExternalInput")
    idx_in = nc.dram_tensor("idx_in", [P, STOT // 16], I16, kind="ExternalInput")
    dstrel_in = nc.dram_tensor("dstrel_in", [P, NT], F32, kind="ExternalInput")
    we_in = nc.dram_tensor("we_in", [P, NT], F32, kind="ExternalInput")

    xd_slice = nc.dram_tensor("xd_slice", [NPAD, P], F32)
    xd_table = nc.dram_tensor("xd_table", [TROWS, P], F32, addr_space="Shared")

    gt_out = nc.dram_tensor("gt_out", [P, NPAD], F32, kind="ExternalOutput")
    sc_out = nc.dram_tensor("sc_out", [P, NW], F32, kind="ExternalOutput")
    dinv_out = nc.dram_tensor("dinv_out", [P, NW], F32, kind="ExternalOutput")

    with tile.TileContext(nc) as tc:
        with (
            tc.tile_pool(name="persist", bufs=1) as pp,
            tc.tile_pool(name="xtiles", bufs=3) as xp,
            tc.tile_pool(name="small", bufs=4) as sp,
            tc.tile_pool(name="gbuf", bufs=3) as gp,
            tc.tile_pool(name="dbuf", bufs=4) as dp,
            tc.tile_pool(name="psum", bufs=4, space="PSUM") as qp,
        ):
            # persistent tiles
            wpad_sb = pp.tile([P, NW * DCAP], F32, tag="wpad")
            nc.sync.dma_start(out=wpad_sb[:], in_=wpad_in[:])
            idx_sb = pp.tile([P, STOT // 16], I16, tag="idx")
            nc.sync.dma_start(out=idx_sb[:], in_=idx_in[:])
            dstrel_sb = pp.tile([P, NT], F32, tag="dstrel")
            nc.sync.dma_start(out=dstrel_sb[:], in_=dstrel_in[:])
            we_sb = pp.tile([P, NT], F32, tag="we")
            nc.sync.dma_start(out=we_sb[:], in_=we_in[:])
            pbc = pp.tile([P, P], F32, tag="pbc")  # p broadcast to all parts
            nc.sync.dma_start(out=pbc[:], in_=p_in[0:1, :].partition_broadcast(P))
            gt_sb = pp.tile([P, NPAD], F32, tag="gt")
            sc_sb = pp.tile([P, NW], F32, tag="sc")
            dinv_sb = pp.tile([P, NW], F32, tag="dinv")
            iota_f = pp.tile([P, P], F32, tag="iota_f")
            nc.sync.dma_start(out=iota_f[:], in_=iota_in[:])

            # ---- prelude per node tile: deg/dinv, scores, Xd
            for t in range(NW):
                xt = xp.tile([P, P], F32, tag="xt")
                nc.sync.dma_start(out=xt[:], in_=X_in[t * P : (t + 1) * P, :])
                deg = sp.tile([P, 1], F32, tag="deg")
                nc.vector.reduce_sum(
                    deg[:], wpad_sb[:, t * DCAP : (t + 1) * DCAP],
                    axis=mybir.AxisListType.X,
                )
                pos = sp.tile([P, 1], F32, tag="pos")
                nc.vector.tensor_scalar(pos[:], deg[:], 0.0, None, ALU.is_gt)
                degc = sp.tile([P, 1], F32, tag="degc")
                nc.vector.tensor_scalar(degc[:], deg[:], 1e-30, None, ALU.max)
                rec = sp.tile([P, 1], F32, tag="rec")
                nc.vector.reciprocal(rec[:], degc[:])
                rs = sp.tile([P, 1], F32, tag="rs")
                nc.scalar.activation(rs[:], rec[:], AF.Sqrt)
                nc.vector.tensor_tensor(
                    out=dinv_sb[:, t : t + 1], in0=rs[:], in1=pos[:], op=ALU.mult
                )
                # scores column t
                tmp = xp.tile([P, P], F32, tag="sctmp")
                nc.vector.tensor_tensor(out=tmp[:], in0=xt[:], in1=pbc[:], op=ALU.mult)
                nc.vector.reduce_sum(
                    sc_sb[:, t : t + 1], tmp[:], axis=mybir.AxisListType.X
                )
                # Xd tile
                xd = xp.tile([P, P], F32, tag="xd")
                nc.vector.tensor_scalar(
                    xd[:], xt[:], dinv_sb[:, t : t + 1], None, ALU.mult
                )
                nc.sync.dma_start(out=xd_slice[t * P : (t + 1) * P, :], in_=xd[:])

            nc.sync.dma_start(out=sc_out[:], in_=sc_sb[:])
            nc.sync.dma_start(out=dinv_out[:], in_=dinv_sb[:])

            # ---- allgather scaled features
            nc.gpsimd.collective_compute(
                "AllGather",
                ALU.bypass,
                replica_groups=[list(range(pl.ncores))],
                ins=[xd_slice[:]],
                outs=[xd_table[:]],
            )

            # ---- gather calls (whole table split in two int16-index halves)
            CB = pl.chunk_blocks
            gtiles = {}  # block index -> (tile, col)
            for ci, (ph, bs, nb) in enumerate(pl.calls):
                g = gp.tile([P, CB, P], F32, tag="g")
                src_ap = (
                    xd_table[0 : min(HALF_SPLIT, TROWS), :]
                    if ph == 0
                    else xd_table[HALF_SPLIT:TROWS, :]
                )
                nc.gpsimd.dma_gather(
                    g[:, 0:nb, :],
                    src_ap,
                    idx_sb[:, bs * 8 : (bs + nb) * 8],
                    nb * P,
                    nb * P,
                    P,
                    queue_num=ci % pl.num_queues,
                    single_packet=False,
                )
                for j in range(nb):
                    gtiles[bs + j] = (g, j)

            # ---- one-hot matmul scatter per window, phases A then B
            def emit_phase(block0, nblocks_list, is_b):
                b = block0
                for w in range(NW):
                    nbk = nblocks_list[w]
                    if nbk == 0:
                        continue
                    ps = qp.tile([P, P], F32, tag="ps")
                    for j in range(nbk):
                        g, col = gtiles[b]
                        d = dp.tile([P, P], F32, tag="d")
                        nc.vector.tensor_scalar(
                            d[:],
                            iota_f[:],
                            dstrel_sb[:, b : b + 1],
                            we_sb[:, b : b + 1],
                            ALU.is_equal,
                            ALU.mult,
                        )
                        nc.tensor.matmul(
                            ps[:],
                            lhsT=g[:, col, :],
                            rhs=d[:],
                            start=(j == 0),
                            stop=(j == nbk - 1),
                        )
                        b += 1
                    dst = gt_sb[:, w * P : (w + 1) * P]
                    if not is_b:
                        nc.scalar.copy(out=dst, in_=ps[:])
                    else:
                        nc.vector.tensor_tensor(out=dst, in0=dst, in1=ps[:], op=ALU.add)
                return b

            nb_a = emit_phase(0, pl.NA, is_b=False)
            assert nb_a == pl.NTA
            nb_b = emit_phase(pl.NTA, pl.NB, is_b=True)
            assert nb_b == pl.NT

            nc.sync.dma_start(out=gt_out[:], in_=gt_sb[:])
    nc.compile()
    return nc


# --------------------------------------------------------------------------
# K2 program: out = dinv * (G @ W_new^T) + b_conv
# --------------------------------------------------------------------------
def build_k2(pl):
    nc = bacc.Bacc(None)
    NPAD, NW = pl.NPAD, pl.NW
    gt_in = nc.dram_tensor("gt_in", [P, NPAD], F32, kind="ExternalInput")
    dinv_in = nc.dram_tensor("dinv_in", [P, NW], F32, kind="ExternalInput")
    wnt_in = nc.dram_tensor("wnt_in", [P, P], F32, kind="ExternalInput")  # W_new^T
    bc_in = nc.dram_tensor("bc_in", [1, P], F32, kind="ExternalInput")
    out = nc.dram_tensor("out", [NPAD, P], F32, kind="ExternalOutput")

    with tile.TileContext(nc) as tc:
        with (
            tc.tile_pool(name="persist", bufs=1) as pp,
            tc.tile_pool(name="work", bufs=4) as wp,
            tc.tile_pool(name="psum", bufs=4, space="PSUM") as qp,
        ):
            gt_sb = pp.tile([P, NPAD], F32, tag="gt")
            nc.sync.dma_start(out=gt_sb[:], in_=gt_in[:])
            dinv_sb = pp.tile([P, NW], F32, tag="dinv")
            nc.sync.dma_start(out=dinv_sb[:], in_=dinv_in[:])
            wnt_sb = pp.tile([P, P], F32, tag="wnt")
            nc.sync.dma_start(out=wnt_sb[:], in_=wnt_in[:])
            bcb = pp.tile([P, P], F32, tag="bcb")
            nc.sync.dma_start(out=bcb[:], in_=bc_in[0:1, :].partition_broadcast(P))

            for t in range(NW):
                ps = qp.tile([P, P], F32, tag="ps")
                nc.tensor.matmul(
                    ps[:],
                    lhsT=gt_sb[:, t * P : (t + 1) * P],
                    rhs=wnt_sb[:],
                    start=True,
                    stop=True,
                )
                sc = wp.tile([P, P], F32, tag="sc")
                nc.vector.tensor_scalar(
                    sc[:], ps[:], dinv_sb[:, t : t + 1], None, ALU.mult
                )
                nc.vector.tensor_tensor(out=sc[:], in0=sc[:], in1=bcb[:], op=ALU.add)
                nc.sync.dma_start(out=out[t * P : (t + 1) * P, :], in_=sc[:])
    nc.compile()
    return nc


# --------------------------------------------------------------------------
# host orchestration
# --------------------------------------------------------------------------
def _gru_wnew(X_tilde, W_ih, W_hh, b_ih, b_hh, W_conv):
    C = W_conv.shape[0]

    def sigmoid(x):
        return 1.0 / (1.0 + np.exp(-x))

    gi = X_tilde @ W_ih.T + b_ih
    gh = W_conv @ W_hh.T + b_hh
    gi_r, gi_z, gi_n = gi[:, :C], gi[:, C : 2 * C], gi[:, 2 * C :]
    gh_r, gh_z, gh_n = gh[:, :C], gh[:, C : 2 * C], gh[:, 2 * C :]
    r = sigmoid(gi_r + gh_r)
    z = sigmoid(gi_z + gh_z)
    n = np.tanh(gi_n + r * gh_n)
    return ((1.0 - z) * n + z * W_conv).astype(np.float32)


def run_pipeline(X, edge_weight, p, W_ih, W_hh, b_ih, b_hh, W_conv, b_conv,
                 edge_index, trace=False, timing=None, plans=None):
    """Full two-kernel device pipeline. Returns (out, results1, results2)."""
    import time as _time

    X = np.asarray(X, np.float32)
    p = np.asarray(p, np.float32)
    pl = plan_shards(X, edge_index, edge_weight)
    if plans is not None:
        plans.append(pl)
    N, C = pl.N, pl.C
    ncores, NPC, NPAD, NW = pl.ncores, pl.NPC, pl.NPAD, pl.NW

    nc1 = build_k1(pl)
    in_maps1 = [
        {
            "X_in": pl.Xp[m],
            "wpad_in": pl.wpad_w[m],
            "p_in": p.reshape(1, C),
            "iota_in": np.broadcast_to(
                np.arange(P, dtype=np.float32), (P, P)
            ).copy(),
            "idx_in": pl.idx_w[m],
            "dstrel_in": pl.dstrel_w[m],
            "we_in": pl.w_w[m],
        }
        for m in range(ncores)
    ]
    t0 = _time.time()
    r1 = run_bass_kernel_spmd(nc1, in_maps1, list(range(ncores)), trace=trace)
    if timing is not None:
        timing["k1_wall_s"] = _time.time() - t0
    res1 = r1.results

    # unwrap scores [P, NW] -> [NPAD] (n = t*128 + p), trim pads, concat
    scores = np.concatenate(
        [res1[m]["sc_out"].T.reshape(NPAD)[:NPC] for m in range(ncores)]
    )
    pnorm = float(np.linalg.norm(p))
    s = scores / pnorm
    k = C
    top_idx = np.argsort(-s, kind="stable")[:k]
    top_vals = s[top_idx]
    X_tilde = X[top_idx] * np.tanh(top_vals)[:, None]
    W_new = _gru_wnew(
        X_tilde.astype(np.float32),
        np.asarray(W_ih, np.float32),
        np.asarray(W_hh, np.float32),
        np.asarray(b_ih, np.float32),
        np.asarray(b_hh, np.float32),
        np.asarray(W_conv, np.float32),
    )

    nc2 = build_k2(pl)
    in_maps2 = [
        {
            "gt_in": res1[m]["gt_out"],
            "dinv_in": res1[m]["dinv_out"],
            "wnt_in": W_new.T.copy(),
            "bc_in": np.asarray(b_conv, np.float32).reshape(1, C),
        }
        for m in range(ncores)
    ]
    t0 = _time.time()
    r2 = run_bass_kernel_spmd(nc2, in_maps2, list(range(ncores)), trace=trace)
    if timing is not None:
        timing["k2_wall_s"] = _time.time() - t0
    res2 = r2.results

    out = np.concatenate([res2[m]["out"][:NPC] for m in range(ncores)], axis=0)
    assert out.shape == (N, C)
    return out.astype(np.float32), r1, r2


def kernel(X, edge_weight, p, W_ih, W_hh, b_ih, b_hh, W_conv, b_conv, edge_index):
    out, _, _ = run_pipeline(
        X, edge_weight, p, W_ih, W_hh, b_ih, b_hh, W_conv, b_conv, edge_index
    )
    return out
